# revision 1
# baseline (speedup 1.0000x reference)
"""Trainium2 Bass kernel for nn_Attention_41686952575399 (sparse attention).

Sharding: data-parallel over batch (2 groups of 4 cores) x tensor-parallel over
heads (4 heads per core). Device-side AllGather of combined heads within each
batch group; each core then computes a 256-wide dout slice of the output
projection for all tokens of its batch element.

Dataflow is fully transposed (features on SBUF partitions, tokens on the free
axis), so attention probabilities come out of the tensor engine already in the
layout the P@V matmul needs and no per-tile transposes are required. Softmax
is computed without max-subtraction (scores*scale is bounded by ~3.2 for this
model's initialization scale) with the denominator fused into the V matmul via
an appended ones-column. All per-head tensors live on partitions 0..63 so
every vector/scalar op is partition-aligned.
"""
import os
import sys

sys.path.insert(0, "/opt/trn_rl_repo")

DEBUG = os.environ.get("BASSK_DEBUG") == "1"

import numpy as np

from concourse import bacc, bass, mybir, tile
from concourse.bass_utils import run_bass_kernel_spmd

B, N, DIM = 2, 1024, 1024
H, DH = 16, 64
WIN, CB = 64, 16
NB = N // CB               # 64 compressed blocks
HPC = 4                    # heads per core
NCORES = 8
GROUPS = [[0, 1, 2, 3], [4, 5, 6, 7]]
F32 = mybir.dt.float32
MM_DT = mybir.dt.float32r  # fast full-precision-ish PE mode
NEG = -1e30
EPS = float(np.finfo(np.float32).eps)
SCALE = float(DH ** -0.5)
NF = 3 * HPC * DH + 3      # 771 projection output features (q,k,v slices + Ws)
KT = NB + 1                # 65: conv block columns + pos-embedding column

AL = mybir.AluOpType
AF = mybir.ActivationFunctionType


def _r(ap):
    """Bitcast a fp32 AP to the matmul dtype (float32r runs the PE at full
    rate for moving dims >= 256)."""
    return ap.bitcast(MM_DT)


def build_program() -> bass.Bass:
    nc = bacc.Bacc("TRN2", target_bir_lowering=False, debug=False,
                   num_devices=NCORES)

    inpT_d = nc.dram_tensor("inpT", [DIM, N], F32, kind="ExternalInput")
    wall_d = nc.dram_tensor("w_all", [DIM, NF], F32, kind="ExternalInput")
    cwk_d = nc.dram_tensor("cw_k", [DH, HPC, CB, DH], F32, kind="ExternalInput")
    cwv_d = nc.dram_tensor("cw_v", [DH, HPC, CB, DH], F32, kind="ExternalInput")
    posk_d = nc.dram_tensor("pos_k", [DH, HPC, CB], F32, kind="ExternalInput")
    posv_d = nc.dram_tensor("pos_v", [DH, HPC, CB], F32, kind="ExternalInput")
    kcb_d = nc.dram_tensor("kcb", [DH, HPC], F32, kind="ExternalInput")
    vcb_d = nc.dram_tensor("vcb", [DH, HPC], F32, kind="ExternalInput")
    bs_d = nc.dram_tensor("bs_t", [3, 1], F32, kind="ExternalInput")
    rms_d = nc.dram_tensor("rms_t", [128, 8], F32, kind="ExternalInput")
    wout_d = nc.dram_tensor("woutS", [128, 8, 256], F32, kind="ExternalInput")
    ones_d = nc.dram_tensor("ones_c", [128, 8], F32, kind="ExternalInput")
    ident_d = nc.dram_tensor("ident_c", [128, 128], F32, kind="ExternalInput")
    outT_d = nc.dram_tensor("outT", [256, N], F32, kind="ExternalOutput")
    dbg = {}
    if DEBUG:
        dbg["s"] = nc.dram_tensor("dbg_s", [1, N], F32, kind="ExternalOutput")
        dbg["w3"] = nc.dram_tensor("dbg_w3", [3, N], F32, kind="ExternalOutput")
        dbg["qkvT"] = nc.dram_tensor("dbg_qkvT", [DH, 12, N + 2 * CB], F32,
                                     kind="ExternalOutput")
        dbg["kbT"] = nc.dram_tensor("dbg_kbT", [DH, CB, KT + 1], F32,
                                    kind="ExternalOutput")
        dbg["ck_f"] = nc.dram_tensor("dbg_ck_f", [DH, NB], F32, kind="ExternalOutput")
        dbg["cv_aug"] = nc.dram_tensor("dbg_cv_aug", [NB, DH + 1], F32,
                                       kind="ExternalOutput")
        dbg["pc"] = nc.dram_tensor("dbg_pc", [NB, N], F32, kind="ExternalOutput")
        dbg["pw"] = nc.dram_tensor("dbg_pw", [128, 8, 256], F32, kind="ExternalOutput")
        dbg["vnat"] = nc.dram_tensor("dbg_vnat", [128, 8, DH + 1], F32,
                                     kind="ExternalOutput")
        dbg["oc"] = nc.dram_tensor("dbg_oc", [DH + 1, N], F32, kind="ExternalOutput")
        dbg["ow"] = nc.dram_tensor("dbg_ow", [DH + 1, N], F32, kind="ExternalOutput")
        dbg["comb"] = nc.dram_tensor("dbg_comb", [DH, HPC, N], F32,
                                     kind="ExternalOutput")
        dbg["cmb"] = nc.dram_tensor("dbg_cmb", [128, 8, N], F32,
                                    kind="ExternalOutput")

    with tile.TileContext(nc) as tc:
        _body(nc, tc, inpT_d, wall_d, cwk_d, cwv_d, posk_d, posv_d,
              kcb_d, vcb_d, bs_d, rms_d, wout_d, outT_d, ones_d, ident_d, dbg)
    nc.compile()
    return nc


def _body(nc, tc, inpT_d, wall_d, cwk_d, cwv_d, posk_d, posv_d,
          kcb_d, vcb_d, bs_d, rms_d, wout_d, outT_d, ones_d, ident_d, dbg):
    mm = nc.tensor.matmul

    # ----- long-lived constants -----------------------------------------
    const_cm = tc.tile_pool(name="const", bufs=1)
    const = const_cm.__enter__()
    ones_col = const.tile([128, 1], F32, name="ones_col")
    ident = const.tile([128, 128], F32, name="ident")
    cmask = const.tile([64, N], F32, name="cmask")
    wmask = const.tile([128, 256], F32, name="wmask")
    rms_sb = const.tile([128, 8], F32, name="rms_sb")
    bs_sb = const.tile([3, 1], F32, name="bs_sb")
    kcb_sb = const.tile([DH, HPC], F32, name="kcb_sb")
    vcb_sb = const.tile([DH, HPC], F32, name="vcb_sb")
    s_row = const.tile([1, N], F32, name="s_row")
    s_tmp = const.tile([1, N], F32, name="s_tmp")
    eps_sb = const.tile([1, 1], F32, name="eps_sb")
    s_bcast = const.tile([128, N], F32, name="s_bcast")
    w3r = const.tile([3, N], F32, name="w3r")
    w3_sb = const.tile([3, N], F32, name="w3_sb")
    w1_row = const.tile([1, N], F32, name="w1_row")
    wout_sb = const.tile([128, 8, 256], F32, name="wout_sb")
    combT = const.tile([DH, HPC, N], F32, name="combT")

    nc.gpsimd.dma_start(out=_r(ones_col[:]), in_=ones_d.ap()[:, 0:1])
    nc.gpsimd.memset(eps_sb[:], EPS)
    nc.gpsimd.dma_start(out=_r(ident[:]), in_=ident_d.ap())
    # compressed-block causal mask: block c visible to token t iff t >= 16c+15
    nc.gpsimd.memset(cmask[:], 0.0)
    nc.gpsimd.affine_select(out=cmask[:], in_=cmask[:], compare_op=AL.is_ge,
                            fill=NEG, base=-15, channel_multiplier=-16,
                            pattern=[[1, N]])
    # window mask on a [key r, query j] tile: visible iff r <= j <= r+63
    nc.gpsimd.memset(wmask[:], 0.0)
    nc.gpsimd.affine_select(out=wmask[:], in_=wmask[:], compare_op=AL.is_ge,
                            fill=NEG, base=0, channel_multiplier=-1,
                            pattern=[[1, 256]])
    nc.gpsimd.affine_select(out=wmask[:], in_=wmask[:], compare_op=AL.is_ge,
                            fill=NEG, base=63, channel_multiplier=1,
                            pattern=[[-1, 256]])

    nc.sync.dma_start(out=rms_sb[:], in_=rms_d.ap())
    nc.sync.dma_start(out=bs_sb[:], in_=bs_d.ap())
    nc.sync.dma_start(out=kcb_sb[:], in_=kcb_d.ap())
    nc.sync.dma_start(out=vcb_sb[:], in_=vcb_d.ap())
    nc.gpsimd.dma_start(out=_r(wout_sb[:]), in_=wout_d.ap())

    # ----- stage 1+2: RMS stats + fused qkv/Ws projection ---------------
    # qkvT column j: 4*part + head (part 0=q, 1=k, 2=v), cols N..N+15 hold
    # the intra-block positional embeddings for the conv's extra column.
    qkvT, qkvT_free = tc.tile([DH, 3 * HPC, N + 2 * CB], F32, name="qkvT")
    x_sb, x_free = tc.tile([128, 8, N], F32, name="x_sb")
    w_sb, w_free = tc.tile([128, 8, NF], F32, name="w_sb")

    for k in range(8):
        nc.gpsimd.dma_start(out=_r(x_sb[:, k, :]), in_=inpT_d.ap()[128 * k:128 * (k + 1), :])
        nc.gpsimd.dma_start(out=_r(w_sb[:, k, :]), in_=wall_d.ap()[128 * k:128 * (k + 1), :])
    nc.gpsimd.dma_start(out=_r(qkvT[:, 4:8, N:N + CB]), in_=posk_d.ap())
    nc.gpsimd.dma_start(out=_r(qkvT[:, 8:12, N:N + CB]), in_=posv_d.ap())
    # fp32r matmuls need an even moving dim: pad the conv with a 66th
    # (zero) block column
    nc.gpsimd.memset(qkvT[:, 4:12, N + CB:N + 2 * CB], 0.0)

    psP_cm = tc.tile_pool(name="psP", bufs=4, space="PSUM")
    psP = psP_cm.__enter__()
    sqp_cm = tc.tile_pool(name="sqp", bufs=2)
    sqp = sqp_cm.__enter__()

    # sum of squares over dim via ones-matmul on squared tiles
    ps_s = [psP.tile([1, 512], F32, name=f"ps_s{ch}", bufs=1) for ch in range(2)]
    for k in range(8):
        sq = sqp.tile([128, N], F32, name="sq")
        if k % 2 == 0:
            nc.scalar.activation(_r(sq[:]), x_sb[:, k, :], AF.Square)
        else:
            nc.vector.tensor_tensor(_r(sq[:]), x_sb[:, k, :], x_sb[:, k, :], op=AL.mult)
        for ch in range(2):
            mm(ps_s[ch][:], _r(ones_col[:]), _r(sq[:, 512 * ch:512 * (ch + 1)]),
               start=(k == 0), stop=(k == 7))
    for ch in range(2):
        nc.scalar.activation(s_tmp[0:1, 512 * ch:512 * (ch + 1)], ps_s[ch][:],
                             AF.Sqrt, bias=eps_sb[:], scale=1.0 / DIM)
    nc.vector.reciprocal(s_row[:], s_tmp[:])
    nc.gpsimd.partition_broadcast(s_bcast[:], s_row[:])

    # fold rms_w into the projection weights (per-partition scalar)
    for k in range(8):
        nc.vector.tensor_scalar(out=_r(w_sb[:, k, :]), in0=w_sb[:, k, :],
                                scalar1=rms_sb[:, k:k + 1], scalar2=None,
                                op0=AL.mult)

    # qkvT[:, j, t] = (W_eff.T @ inpT)[feat, t] * s[t]; psum rows 64..127
    # belong to the odd head of the feature tile and are moved down to
    # partitions 0..63 via a partition-shifting SBUF->SBUF DMA.
    for f in range(7):
        for ch in range(2):
            pp = psP.tile([128, 512], F32, name="pp")
            sl = slice(512 * ch, 512 * (ch + 1))
            M = 128 if f < 6 else 3
            for k in range(8):
                mm(pp[:M, :], _r(w_sb[:, k, 128 * f:128 * f + M]),
                   _r(x_sb[:, k, sl]), start=(k == 0), stop=(k == 7))
            if f < 6:
                jA = 4 * (f // 2) + 2 * (f % 2)
                nc.vector.tensor_tensor(_r(qkvT[:, jA, sl]), pp[0:64, :],
                                        s_bcast[0:64, sl], op=AL.mult)
                stage = sqp.tile([128, 512], F32, name="stage")
                nc.vector.tensor_tensor(_r(stage[64:128, :]), pp[64:128, :],
                                        s_bcast[64:128, sl], op=AL.mult)
                nc.sync.dma_start(out=_r(qkvT[:, jA + 1, sl]),
                                  in_=_r(stage[64:128, :]))
            else:
                nc.vector.tensor_tensor(w3r[:, sl], pp[:3, :],
                                        s_bcast[:3, sl], op=AL.mult)
    nc.scalar.activation(w3_sb[:], w3r[:], AF.Sigmoid, bias=bs_sb[:])
    if DEBUG:
        nc.sync.dma_start(out=dbg["s"].ap(), in_=s_row[:])
        nc.sync.dma_start(out=dbg["w3"].ap(), in_=w3_sb[:])
    nc.sync.dma_start(out=w1_row[:], in_=w3_sb[1:2, :])

    sqp_cm.__exit__(None, None, None)
    psP_cm.__exit__(None, None, None)
    w_free()
    x_free()

    # ----- stage 3-6: per-head attention --------------------------------
    cwp_cm = tc.tile_pool(name="cwp", bufs=1)
    cwp = cwp_cm.__enter__()
    cwk_sb = cwp.tile([DH, HPC, CB, DH], F32, name="cwk_sb")
    cwv_sb = cwp.tile([DH, HPC, CB, DH], F32, name="cwv_sb")
    nc.gpsimd.dma_start(out=_r(cwk_sb[:]), in_=cwk_d.ap())
    nc.gpsimd.dma_start(out=_r(cwv_sb[:]), in_=cwv_d.ap())

    psA_cm = tc.tile_pool(name="psA", bufs=3, space="PSUM")
    psA = psA_cm.__enter__()
    psO_cm = tc.tile_pool(name="psO", bufs=1, space="PSUM")
    psO = psO_cm.__enter__()
    pat_cm = tc.tile_pool(name="attn", bufs=1)
    pat = pat_cm.__enter__()
    pat2_cm = tc.tile_pool(name="attn2", bufs=2)
    pat2 = pat2_cm.__enter__()

    for h in range(HPC):
        qT = qkvT[:, h, 0:N]
        kTp = qkvT[:, 4 + h, :].rearrange("p (c t) -> p t c", t=CB)
        vTp = qkvT[:, 8 + h, :].rearrange("p (c t) -> p t c", t=CB)
        kT = qkvT[:, 4 + h, 0:N]
        vT = qkvT[:, 8 + h, 0:N]

        # -- compression conv: ckT[o,c] / cv[c,o]; c=NB is the pos column --
        # de-interleave tokens-within-block to the middle axis so each
        # per-t matmul reads a contiguous [64, 65] slab
        kbT = pat2.tile([DH, CB, KT + 1], F32, name="kbT", bufs=1)
        nc.vector.tensor_copy(_r(kbT[:]), kTp)
        vbT = pat2.tile([DH, CB, KT + 1], F32, name="vbT", bufs=1)
        nc.scalar.copy(_r(vbT[:]), vTp)

        ps_ck = psA.tile([DH, KT + 1], F32, name="ps_ck", tag="psa")
        for t in range(CB):
            mm(ps_ck[:], _r(cwk_sb[:, h, t, :]), _r(kbT[:, t, :]),
               start=(t == 0), stop=(t == CB - 1))
        ck_sb = pat2.tile([DH, KT + 1], F32, name="ck_sb", bufs=1)
        nc.scalar.copy(ck_sb[:], ps_ck[:])
        ck_f = pat2.tile([DH, NB], F32, name="ck_f")
        nc.vector.tensor_scalar(out=_r(ck_f[:]), in0=ck_sb[:, 0:NB],
                                scalar1=ck_sb[:, NB:NB + 1],
                                scalar2=kcb_sb[:, h:h + 1],
                                op0=AL.add, op1=AL.add)

        ps_cv = psA.tile([DH, KT + 1], F32, name="ps_cv", tag="psa")
        for t in range(CB):
            mm(ps_cv[:], _r(cwv_sb[:, h, t, :]), _r(vbT[:, t, :]),
               start=(t == 0), stop=(t == CB - 1))
        cv_sb = pat2.tile([DH, KT + 1], F32, name="cv_sb", bufs=1)
        nc.scalar.copy(cv_sb[:], ps_cv[:])
        cvT_f = pat2.tile([DH, NB], F32, name="cvT_f")
        nc.vector.tensor_scalar(out=_r(cvT_f[:]), in0=cv_sb[:, 0:NB],
                                scalar1=cv_sb[:, NB:NB + 1],
                                scalar2=vcb_sb[:, h:h + 1],
                                op0=AL.add, op1=AL.add)
        # natural [block, dh] orientation with a leading ones column so the
        # AV matmul emits the softmax denominator on partition 0
        ps_cvt = psA.tile([NB, DH], F32, name="ps_cvt", tag="psa")
        nc.tensor.transpose(_r(ps_cvt[:]), _r(cvT_f[:]), _r(ident[0:64, 0:64]))
        cv_aug = pat2.tile([NB, DH + 1], F32, name="cv_aug")
        nc.scalar.copy(_r(cv_aug[:, 0:DH]), ps_cvt[:])
        nc.gpsimd.dma_start(out=_r(cv_aug[:, DH:DH + 1]),
                            in_=ones_d.ap()[0:64, 0:1])

        # -- compressed branch: ScT [c,t] -> exp -> (cv_aug).T @ P --------
        pc = pat.tile([NB, N], F32, name="pc")
        ps_oc = [psO.tile([DH + 1, 512], F32, name=f"ps_oc{ch}") for ch in range(2)]
        for ch in range(2):
            sl = slice(512 * ch, 512 * (ch + 1))
            ps_sc = psA.tile([NB, 512], F32, name="ps_sc", tag="psa")
            mm(ps_sc[:], _r(ck_f[:]), _r(qT[:, sl]), start=True, stop=True)
            nc.vector.tensor_tensor(ps_sc[:], ps_sc[:], cmask[:, sl], op=AL.add)
            nc.scalar.activation(_r(pc[:, sl]), ps_sc[:], AF.Exp, scale=SCALE)
            mm(ps_oc[ch][:], _r(cv_aug[:]), _r(pc[:, sl]), start=True, stop=True)

        # -- sliding window branch: SwT [key r, query j] per key tile -----
        pw = pat.tile([128, 8, 256], F32, name="pw")
        for kt in range(8):
            nq = 256 if kt < 7 else 128
            ps_sw = psA.tile([128, 256], F32, name="ps_sw", tag="psa")
            mm(ps_sw[:, :nq], _r(kT[:, 128 * kt:128 * (kt + 1)]),
               _r(qT[:, 128 * kt:128 * kt + nq]), start=True, stop=True)
            nc.vector.tensor_tensor(ps_sw[:, :nq], ps_sw[:, :nq], wmask[:, :nq],
                                    op=AL.add)
            nc.scalar.activation(_r(pw[:, kt, :nq]), ps_sw[:, :nq], AF.Exp,
                                 scale=SCALE)

        # v in natural [token, dh] layout + ones column (via PE transpose)
        vnat = pat.tile([128, 8, DH + 1], F32, name="vnat")
        for g in range(8):
            ps_vt = psA.tile([128, DH], F32, name="ps_vt", tag="psa")
            nc.tensor.transpose(_r(ps_vt[:]), _r(vT[:, 128 * g:128 * (g + 1)]),
                                _r(ident[0:64, 0:64]))
            nc.scalar.copy(_r(vnat[:, g, 0:DH]), ps_vt[:])
        nc.gpsimd.dma_start(out=_r(vnat[:, :, DH:DH + 1]),
                            in_=ones_d.ap()[:, 0:8])

        ps_ow = [psO.tile([DH + 1, 512], F32, name=f"ps_ow{ch}") for ch in range(2)]
        for qt in range(8):
            dst = ps_ow[qt // 4][:, (qt % 4) * 128:(qt % 4) * 128 + 128]
            if qt == 0:
                mm(dst, _r(vnat[:, 0, :]), _r(pw[:, 0, 0:128]),
                   start=True, stop=True)
            else:
                mm(dst, _r(vnat[:, qt - 1, :]), _r(pw[:, qt - 1, 128:256]),
                   start=True, stop=False)
                mm(dst, _r(vnat[:, qt, :]), _r(pw[:, qt, 0:128]),
                   start=False, stop=True)

        # -- mix the two branches with the learned gates ------------------
        # reciprocal of the fused denominators (rows at partition 64 of
        # the psum outputs), then DMA-shift the result rows to partition 0
        # (HW partition_broadcast always reads the tile's partition 0)
        sc64 = pat.tile([65, N], F32, name="sc64")
        sw64 = pat.tile([65, N], F32, name="sw64")
        for ch in range(2):
            sl = slice(512 * ch, 512 * (ch + 1))
            nc.vector.reciprocal(sc64[64:65, sl], ps_oc[ch][DH:DH + 1, :])
            nc.vector.reciprocal(sw64[64:65, sl], ps_ow[ch][DH:DH + 1, :])
        sc_row = pat.tile([1, N], F32, name="sc_row")
        sw_row = pat.tile([1, N], F32, name="sw_row")
        nc.sync.dma_start(out=sc_row[:], in_=sc64[64:65, :])
        nc.sync.dma_start(out=sw_row[:], in_=sw64[64:65, :])
        nc.vector.tensor_tensor(sc_row[:], sc_row[:], w3_sb[0:1, :], op=AL.mult)
        nc.vector.tensor_tensor(sw_row[:], sw_row[:], w1_row[:], op=AL.mult)
        # tokens 0..14 see no compressed block: den==0 -> force gate to 0
        nc.vector.memset(sc_row[0:1, 0:15], 0.0)
        sc_b = pat.tile([DH, N], F32, name="sc_b")
        sw_b = pat.tile([DH, N], F32, name="sw_b")
        nc.gpsimd.partition_broadcast(sc_b[:], sc_row[:])
        nc.gpsimd.partition_broadcast(sw_b[:], sw_row[:])
        mixt = pat.tile([DH, N], F32, name="mixt")
        for ch in range(2):
            sl = slice(512 * ch, 512 * (ch + 1))
            nc.vector.tensor_tensor(mixt[:, sl], ps_oc[ch][0:DH, :],
                                    sc_b[:, sl], op=AL.mult)
            nc.vector.tensor_tensor(combT[:, h, sl], ps_ow[ch][0:DH, :],
                                    sw_b[:, sl], op=AL.mult)
            nc.vector.tensor_tensor(combT[:, h, sl], combT[:, h, sl],
                                    mixt[:, sl], op=AL.add)
        if DEBUG and h == 0:
            nc.sync.dma_start(out=dbg["qkvT"].ap(), in_=qkvT[:])
            nc.sync.dma_start(out=dbg["kbT"].ap(), in_=kbT[:])
            nc.sync.dma_start(out=dbg["ck_f"].ap(), in_=ck_f[:])
            nc.sync.dma_start(out=dbg["cv_aug"].ap(), in_=cv_aug[:])
            nc.sync.dma_start(out=dbg["pc"].ap(), in_=pc[:])
            nc.sync.dma_start(out=dbg["pw"].ap(), in_=pw[:])
            nc.sync.dma_start(out=dbg["vnat"].ap(), in_=vnat[:])
            dbg_oc_sb = pat2.tile([DH + 1, N], F32, name="dbg_oc_sb", bufs=1)
            dbg_ow_sb = pat2.tile([DH + 1, N], F32, name="dbg_ow_sb", bufs=1)
            for ch in range(2):
                sl = slice(512 * ch, 512 * (ch + 1))
                nc.scalar.copy(dbg_oc_sb[:, sl], ps_oc[ch][:])
                nc.scalar.copy(dbg_ow_sb[:, sl], ps_ow[ch][:])
            nc.sync.dma_start(out=dbg["oc"].ap(), in_=dbg_oc_sb[:])
            nc.sync.dma_start(out=dbg["ow"].ap(), in_=dbg_ow_sb[:])

    pat2_cm.__exit__(None, None, None)
    pat_cm.__exit__(None, None, None)
    psO_cm.__exit__(None, None, None)
    psA_cm.__exit__(None, None, None)
    cwp_cm.__exit__(None, None, None)
    qkvT_free()

    # ----- stage 7: AllGather heads within batch group + output proj ----
    dram_cm = tc.tile_pool(name="dram", bufs=1, space="DRAM")
    dram = dram_cm.__enter__()
    cc_in = dram.tile([HPC * DH, N], F32, name="cc_in")
    cc_out = dram.tile([4 * HPC * DH, N], F32, name="cc_out")

    if DEBUG:
        nc.sync.dma_start(out=dbg["comb"].ap(), in_=combT[:])
    nc.sync.dma_start(out=cc_in[:].rearrange("(hh p) n -> p hh n", p=64),
                      in_=combT[:])
    nc.gpsimd.collective_compute(
        "AllGather", AL.bypass, replica_groups=GROUPS,
        ins=[cc_in[:].opt()], outs=[cc_out[:].opt()])

    cmb_sb, cmb_free = tc.tile([128, 8, N], F32, name="cmb_sb")
    outT_sb, outT_sb_free = tc.tile([128, 2, N], F32, name="outT_sb")
    for k in range(8):
        nc.gpsimd.dma_start(out=_r(cmb_sb[:, k, :]),
                          in_=cc_out[128 * k:128 * (k + 1), :])

    if DEBUG:
        nc.sync.dma_start(out=dbg["cmb"].ap(), in_=cmb_sb[:])
    psW_cm = tc.tile_pool(name="psW", bufs=4, space="PSUM")
    psW = psW_cm.__enter__()
    for m in range(2):
        for ch in range(2):
            sl = slice(512 * ch, 512 * (ch + 1))
            po = psW.tile([128, 512], F32, name="po")
            for k in range(8):
                mm(po[:], _r(wout_sb[:, k, 128 * m:128 * (m + 1)]),
                   _r(cmb_sb[:, k, sl]), start=(k == 0), stop=(k == 7))
            nc.scalar.copy(outT_sb[:, m, sl], po[:])
    nc.sync.dma_start(out=outT_d.ap().rearrange("(m p) n -> p m n", p=128),
                      in_=outT_sb[:])

    psW_cm.__exit__(None, None, None)
    outT_sb_free()
    cmb_free()
    dram_cm.__exit__(None, None, None)
    const_cm.__exit__(None, None, None)


# --------------------------------------------------------------------------
_CACHE: dict = {}


def _get_nc() -> bass.Bass:
    if "nc" not in _CACHE:
        _CACHE["nc"] = build_program()
    return _CACHE["nc"]


def _prep_core(c: int, inputs: dict) -> dict:
    b, r = c // 4, c % 4
    hs = HPC * r
    f32 = np.float32
    inp = np.asarray(inputs["inp"], f32)
    rms_w = np.asarray(inputs["rms_w"], f32)
    Wqkv = np.asarray(inputs["Wqkv"], f32)
    k_pos = np.asarray(inputs["k_pos"], f32)
    v_pos = np.asarray(inputs["v_pos"], f32)
    k_cw = np.asarray(inputs["k_cw"], f32)
    k_cb = np.asarray(inputs["k_cb"], f32)
    v_cw = np.asarray(inputs["v_cw"], f32)
    v_cb = np.asarray(inputs["v_cb"], f32)
    Ws = np.asarray(inputs["Ws"], f32)
    bs = np.asarray(inputs["bs"], f32)
    Wout = np.asarray(inputs["Wout"], f32)

    cols = [Wqkv[:, p * H * DH + hs * DH: p * H * DH + (hs + HPC) * DH]
            for p in range(3)]
    w_all = np.ascontiguousarray(np.concatenate(cols + [Ws], axis=1))

    return {
        "inpT": np.ascontiguousarray(inp[b].T),
        "w_all": w_all,
        # [i, h, t, o] = cw[hs+h, o, i, t]
        "cw_k": np.ascontiguousarray(k_cw[hs:hs + HPC].transpose(2, 0, 3, 1)),
        "cw_v": np.ascontiguousarray(v_cw[hs:hs + HPC].transpose(2, 0, 3, 1)),
        # [i, h, t] = pos[hs+h, t, i]
        "pos_k": np.ascontiguousarray(k_pos[hs:hs + HPC].transpose(2, 0, 1)),
        "pos_v": np.ascontiguousarray(v_pos[hs:hs + HPC].transpose(2, 0, 1)),
        "kcb": np.ascontiguousarray(k_cb[hs:hs + HPC].T),
        "vcb": np.ascontiguousarray(v_cb[hs:hs + HPC].T),
        "bs_t": np.ascontiguousarray(bs[:, None]),
        "rms_t": np.ascontiguousarray(rms_w.reshape(8, 128).T),
        "woutS": np.ascontiguousarray(
            Wout[:, 256 * r:256 * (r + 1)].reshape(8, 128, 256).transpose(1, 0, 2)),
        "ones_c": np.ones((128, 8), f32),
        "ident_c": np.eye(128, dtype=f32),
    }


def kernel(**inputs) -> np.ndarray:
    nc = _get_nc()
    in_maps = [_prep_core(c, inputs) for c in range(NCORES)]
    res = run_bass_kernel_spmd(nc, in_maps, list(range(NCORES)))
    out = np.zeros((B, N, DIM), np.float32)
    for c in range(NCORES):
        b, r = c // 4, c % 4
        out[b, :, 256 * r:256 * (r + 1)] = res.results[c]["outT"].T
    return out



# revision 23
# speedup vs baseline: 1.6660x; 1.6660x over previous
"""Trainium2 Bass kernel for nn_Attention_41686952575399 (sparse attention).

Sharding: data-parallel over batch (2 groups of 4 cores) x tensor-parallel over
heads (4 heads per core). Per-head device-side AllGather chunks (bf16) overlap
with the next head's compute; each core then computes a 256-wide dout slice of
the output projection for all tokens of its batch element.

vs the fp32r baseline:
  - bf16 datapath for all matmul operands (psum accumulation stays fp32);
    halves HBM loads and collective bytes.
  - rms_w / conv positional embedding / conv bias folded on the host.
  - compression conv as 8 accumulating matmuls with (dh, token-parity) packed
    128-partition contraction instead of 16 64-contraction matmuls.
  - window P@V via 8 matmuls into two 256-col-span psum accumulators (even /
    odd key tiles, odd spans shifted 128 cols) instead of 15 128-col matmuls.
  - paired PE transposes produce V in natural layout for two heads at once.
  - reciprocal_approx_fast for softmax denominators and RMS norm.
  - per-head AllGather issued right after each head's mix -> only the last
    ~quarter of the collective is exposed.
"""
import os
import sys

sys.path.insert(0, "/opt/trn_rl_repo")

DEBUG = os.environ.get("BASSK_DEBUG") == "1"

import numpy as np
import ml_dtypes

from concourse import bacc, bass, mybir, tile
from concourse.bass_utils import run_bass_kernel_spmd

B, N, DIM = 2, 1024, 1024
H, DH = 16, 64
WIN, CB = 64, 16
NB = N // CB               # 64 compressed blocks
HPC = 4                    # heads per core
NCORES = 8
GROUPS = [[0, 1, 2, 3], [4, 5, 6, 7]]
F32 = mybir.dt.float32
BF16 = mybir.dt.bfloat16
NEG = -1e30
EPS = float(np.finfo(np.float32).eps)
SCALE = float(DH ** -0.5)
NF = 3 * HPC * DH + 3      # 771 projection output features (q,k,v slices + Ws)
NPB = ml_dtypes.bfloat16

AL = mybir.AluOpType
AF = mybir.ActivationFunctionType


def build_program() -> bass.Bass:
    nc = bacc.Bacc("TRN2", target_bir_lowering=False, debug=False,
                   num_devices=NCORES)

    inpT_d = nc.dram_tensor("inpT", [DIM, N], BF16, kind="ExternalInput")
    wall_d = nc.dram_tensor("w_all", [DIM, NF], BF16, kind="ExternalInput")
    cwk_d = nc.dram_tensor("cw_k", [128, 2, CB, 128], BF16, kind="ExternalInput")
    cwv_d = nc.dram_tensor("cw_v", [128, 2, CB, 128], BF16, kind="ExternalInput")
    kcb_d = nc.dram_tensor("kcb", [128, 2], F32, kind="ExternalInput")
    vcb_d = nc.dram_tensor("vcb", [128, 2], F32, kind="ExternalInput")
    bs_d = nc.dram_tensor("bs_t", [3, 1], F32, kind="ExternalInput")
    wout_d = nc.dram_tensor("woutS", [128, HPC, 2, 256], BF16, kind="ExternalInput")
    ones_d = nc.dram_tensor("ones_c", [128, 8], BF16, kind="ExternalInput")
    ident_d = nc.dram_tensor("ident_c", [128, 128], BF16, kind="ExternalInput")
    outT_d = nc.dram_tensor("outT", [256, N], F32, kind="ExternalOutput")
    dbg = {}
    if DEBUG:
        dbg["s"] = nc.dram_tensor("dbg_s", [1, N], F32, kind="ExternalOutput")
        dbg["w3"] = nc.dram_tensor("dbg_w3", [3, N], F32, kind="ExternalOutput")
        dbg["qk"] = nc.dram_tensor("dbg_qk", [128, 2, 2, N], BF16,
                                   kind="ExternalOutput")
        dbg["v2"] = nc.dram_tensor("dbg_v2", [128, 2, N], BF16,
                                   kind="ExternalOutput")
        dbg["ck"] = nc.dram_tensor("dbg_ck", [128, DH], BF16, kind="ExternalOutput")
        dbg["cva"] = nc.dram_tensor("dbg_cva", [DH, DH + 1], BF16,
                                    kind="ExternalOutput")
        dbg["pc"] = nc.dram_tensor("dbg_pc", [NB, 2, N], BF16, kind="ExternalOutput")
        dbg["pw"] = nc.dram_tensor("dbg_pw", [128, 2, 8, 256], BF16,
                                   kind="ExternalOutput")
        dbg["vnat"] = nc.dram_tensor("dbg_vnat", [128, 2, 8, 130], BF16,
                                     kind="ExternalOutput")
        dbg["den"] = nc.dram_tensor("dbg_den", [2, N], F32, kind="ExternalOutput")
        dbg["denr"] = nc.dram_tensor("dbg_denr", [2, N], F32, kind="ExternalOutput")
        dbg["comb"] = nc.dram_tensor("dbg_comb", [DH, HPC, N], BF16,
                                     kind="ExternalOutput")
        dbg["cmb"] = nc.dram_tensor("dbg_cmb", [128, HPC, 2, N], BF16,
                                    kind="ExternalOutput")

    with tile.TileContext(nc) as tc:
        _body(nc, tc, inpT_d, wall_d, cwk_d, cwv_d, kcb_d, vcb_d, bs_d,
              wout_d, ones_d, ident_d, outT_d, dbg)
    nc.compile()
    return nc


def _body(nc, tc, inpT_d, wall_d, cwk_d, cwv_d, kcb_d, vcb_d, bs_d,
          wout_d, ones_d, ident_d, outT_d, dbg):
    mm = nc.tensor.matmul

    # ----- long-lived constants -----------------------------------------
    const_cm = tc.tile_pool(name="const", bufs=1)
    const = const_cm.__enter__()
    ones_b = const.tile([128, 8], BF16, name="ones_b")
    ident = const.tile([128, 128], BF16, name="ident")
    cmask = const.tile([NB, N], F32, name="cmask")
    wmask = const.tile([128, 192], F32, name="wmask")
    bs_sb = const.tile([3, 1], F32, name="bs_sb")
    kcb_sb = const.tile([128, 2], F32, name="kcb_sb")
    vcb_sb = const.tile([128, 2], F32, name="vcb_sb")
    eps_sb = const.tile([1, 1], F32, name="eps_sb")
    s_srt = const.tile([1, N], F32, name="s_srt")
    s_row = const.tile([1, N], F32, name="s_row")
    s_bcast = const.tile([128, N], F32, name="s_bcast")
    w3r = const.tile([3, N], F32, name="w3r")
    w3_sb = const.tile([3, N], F32, name="w3_sb")
    wout_sb = const.tile([128, HPC, 2, 256], BF16, name="wout_sb")
    cwk_sb = const.tile([128, 2, CB, 128], BF16, name="cwk_sb")
    cwv_sb = const.tile([128, 2, CB, 128], BF16, name="cwv_sb")

    nc.gpsimd.dma_start(out=ones_b[:], in_=ones_d.ap())
    nc.gpsimd.dma_start(out=ident[:], in_=ident_d.ap())
    nc.gpsimd.memset(eps_sb[:], EPS)
    # compressed-block causal mask: block c visible to token t iff t >= 16c+15
    nc.gpsimd.memset(cmask[:], 0.0)
    nc.gpsimd.affine_select(out=cmask[:], in_=cmask[:], compare_op=AL.is_ge,
                            fill=NEG, base=-15, channel_multiplier=-16,
                            pattern=[[1, N]])
    # window mask on a [key r, query j] tile: visible iff r <= j <= r+63
    nc.gpsimd.memset(wmask[:], 0.0)
    nc.gpsimd.affine_select(out=wmask[:], in_=wmask[:], compare_op=AL.is_ge,
                            fill=NEG, base=0, channel_multiplier=-1,
                            pattern=[[1, 192]])
    nc.gpsimd.affine_select(out=wmask[:], in_=wmask[:], compare_op=AL.is_ge,
                            fill=NEG, base=63, channel_multiplier=1,
                            pattern=[[-1, 192]])

    nc.sync.dma_start(out=bs_sb[:], in_=bs_d.ap())
    nc.sync.dma_start(out=kcb_sb[:], in_=kcb_d.ap())
    nc.sync.dma_start(out=vcb_sb[:], in_=vcb_d.ap())
    nc.gpsimd.dma_start(out=wout_sb[:], in_=wout_d.ap())
    nc.gpsimd.dma_start(out=cwk_sb[:], in_=cwk_d.ap())
    nc.gpsimd.dma_start(out=cwv_sb[:], in_=cwv_d.ap())

    # ----- stage 1+2: RMS stats + fused qkv/Ws projection ---------------
    # qk2: q/k with even head of the pair on partitions 0-63, odd on 64-127.
    qk2, qk2_free = tc.tile([128, 2, 2, N], BF16, name="qk2")
    v2, v2_free = tc.tile([128, 2, N], BF16, name="v2")
    q_od, q_od_free = tc.tile([DH, 2, N], BF16, name="q_od")
    k_od, k_od_free = tc.tile([DH, 2, N], BF16, name="k_od")
    x_sb, x_free = tc.tile([128, 8, N], BF16, name="x_sb")
    w_sb, w_free = tc.tile([128, 8, NF], BF16, name="w_sb")

    for k in range(8):
        nc.gpsimd.dma_start(out=x_sb[:, k, :], in_=inpT_d.ap()[128 * k:128 * (k + 1), :])
        nc.gpsimd.dma_start(out=w_sb[:, k, :], in_=wall_d.ap()[128 * k:128 * (k + 1), :])

    psP_cm = tc.tile_pool(name="psP", bufs=4, space="PSUM")
    psP = psP_cm.__enter__()
    sqp_cm = tc.tile_pool(name="sqp", bufs=2)
    sqp = sqp_cm.__enter__()

    # sum of squares over dim via ones-matmul on squared tiles
    ps_s = [psP.tile([1, 512], F32, name=f"ps_s{ch}", bufs=1) for ch in range(2)]
    for k in range(8):
        sq = sqp.tile([128, N], BF16, name="sq")
        if k % 2 == 0:
            nc.scalar.activation(sq[:], x_sb[:, k, :], AF.Square)
        else:
            nc.vector.tensor_tensor(sq[:], x_sb[:, k, :], x_sb[:, k, :], op=AL.mult)
        for ch in range(2):
            mm(ps_s[ch][:], ones_b[:, 0:1], sq[:, 512 * ch:512 * (ch + 1)],
               start=(k == 0), stop=(k == 7))
    for ch in range(2):
        nc.scalar.activation(s_srt[0:1, 512 * ch:512 * (ch + 1)], ps_s[ch][:],
                             AF.Sqrt, bias=eps_sb[:], scale=1.0 / DIM)
    nc.vector.reciprocal_approx_fast(out=s_row[:], in_=s_srt[:])
    nc.gpsimd.partition_broadcast(s_bcast[:], s_row[:])

    # qkv projection: f-tile layout [q01 | q23 | k01 | k23 | v01 | v23 | Ws]
    for f in range(7):
        for ch in range(2):
            pp = psP.tile([128, 512], F32, name="pp")
            sl = slice(512 * ch, 512 * (ch + 1))
            M = 128 if f < 6 else 3
            for k in range(8):
                mm(pp[:M, :], w_sb[:, k, 128 * f:128 * f + M],
                   x_sb[:, k, sl], start=(k == 0), stop=(k == 7))
            if f < 6:
                kind, hp = f // 2, f % 2
                if kind < 2:
                    nc.vector.tensor_tensor(qk2[:, hp, kind, sl], pp[:, :],
                                            s_bcast[:, sl], op=AL.mult)
                else:
                    nc.vector.tensor_tensor(v2[:, hp, sl], pp[:, :],
                                            s_bcast[:, sl], op=AL.mult)
            else:
                nc.vector.tensor_tensor(w3r[:, sl], pp[:3, :],
                                        s_bcast[:3, sl], op=AL.mult)
    nc.scalar.activation(w3_sb[:], w3r[:], AF.Sigmoid, bias=bs_sb[:])
    # odd heads' q/k shifted down to partitions 0-63
    for hp in range(2):
        nc.sync.dma_start(out=q_od[:, hp, :], in_=qk2[64:128, hp, 0, :])
        nc.sync.dma_start(out=k_od[:, hp, :], in_=qk2[64:128, hp, 1, :])
    if DEBUG:
        nc.sync.dma_start(out=dbg["s"].ap(), in_=s_row[:])
        nc.sync.dma_start(out=dbg["w3"].ap(), in_=w3_sb[:])
        nc.sync.dma_start(out=dbg["qk"].ap(), in_=qk2[:])
        nc.sync.dma_start(out=dbg["v2"].ap(), in_=v2[:])

    sqp_cm.__exit__(None, None, None)
    psP_cm.__exit__(None, None, None)
    w_free()
    x_free()

    # ----- stage 3-6: per-head attention --------------------------------
    att_cm = tc.tile_pool(name="att", bufs=1)
    att = att_cm.__enter__()
    ck_f2 = att.tile([128, 2, DH], BF16, name="ck_f2")
    cv_f2 = att.tile([128, 2, DH], BF16, name="cv_f2")
    ck_lo = att.tile([DH, 2, DH], BF16, name="ck_lo")
    cv_aug = att.tile([DH, 2, DH + 1], BF16, name="cv_aug")
    pc = att.tile([NB, 2, N], BF16, name="pc")
    pw = att.tile([128, 2, 8, 256], BF16, name="pw")
    vnat2 = att.tile([128, 2, 8, 130], BF16, name="vnat2")
    comb = att.tile([DH, HPC, N], BF16, name="comb")
    t1 = att.tile([DH, N], F32, name="t1")
    t2 = att.tile([DH, N], F32, name="t2")
    t3 = att.tile([DH, 896], F32, name="t3")
    dw64 = att.tile([65, N], F32, name="dw64")
    dwbs = att.tile([65, 896], F32, name="dwbs")
    dc64 = att.tile([65, N], F32, name="dc64")
    d2 = att.tile([2, N], F32, name="d2")
    r2 = att.tile([2, N], F32, name="r2")
    g2 = att.tile([2, N], F32, name="g2")
    gw_row = att.tile([1, N], F32, name="gw_row")
    gc_b = att.tile([DH, N], F32, name="gc_b")
    gw_b = att.tile([DH, N], F32, name="gw_b")

    # static pieces: ones columns for the AV denominators, zero pads for the
    # 192:256 query-span tails of the window probability tiles
    nc.gpsimd.memset(cv_aug[:, :, DH:DH + 1], 1.0)
    nc.gpsimd.memset(vnat2[:, :, :, 64:65], 1.0)
    nc.gpsimd.memset(vnat2[:, :, :, 129:130], 1.0)
    nc.gpsimd.memset(pw[:, :, :, 192:256], 0.0)

    psA_cm = tc.tile_pool(name="psA", bufs=2, space="PSUM")
    psA = psA_cm.__enter__()
    psO_cm = tc.tile_pool(name="psO", bufs=1, space="PSUM")
    psO = psO_cm.__enter__()

    dram_cm = tc.tile_pool(name="dram", bufs=1, space="DRAM")
    dram = dram_cm.__enter__()
    cc_in = dram.tile([HPC * DH, N], BF16, name="cc_in")
    cc_out = dram.tile([4 * HPC * DH, N], BF16, name="cc_out")

    for h in range(HPC):
        hp, par = h // 2, h % 2
        hb = par
        if par == 0:
            # V natural layout for both heads of the pair, via paired
            # [128,128] PE transposes; col 64 / 129 hold the ones columns.
            for g in range(8):
                ps_vt = psA.tile([128, 128], BF16, name="ps_vt", tag="psa")
                nc.tensor.transpose(ps_vt[:], v2[:, hp, 128 * g:128 * (g + 1)],
                                    ident[:, 0:128])
                nc.scalar.copy(vnat2[:, hp, g, 0:64], ps_vt[:, 0:64])
                nc.scalar.copy(vnat2[:, hp, g, 65:129], ps_vt[:, 64:128])

            # -- compression conv, both heads at once: contraction over the
            # pair-stacked 128 partitions with block-diagonal weights; the
            # moving operand reads block-strided columns of k/v in place.
            kmv = qk2[:, hp, 1, :].rearrange("p (c t) -> p t c", t=CB)
            vmv = v2[:, hp, :].rearrange("p (c t) -> p t c", t=CB)
            ps_ck = psA.tile([128, DH], F32, name="ps_ck", tag="psa")
            for t in range(CB):
                mm(ps_ck[:], cwk_sb[:, hp, t, :], kmv[:, t, :],
                   start=(t == 0), stop=(t == CB - 1))
            nc.scalar.activation(ck_f2[:, hp, :], ps_ck[:], AF.Identity,
                                 bias=kcb_sb[:, hp:hp + 1])
            ps_cv = psA.tile([128, DH], F32, name="ps_cv", tag="psa")
            for t in range(CB):
                mm(ps_cv[:], cwv_sb[:, hp, t, :], vmv[:, t, :],
                   start=(t == 0), stop=(t == CB - 1))
            nc.scalar.activation(cv_f2[:, hp, :], ps_cv[:], AF.Identity,
                                 bias=vcb_sb[:, hp:hp + 1])

        if par == 0:
            qT = qk2[0:64, hp, 0, :]
            kT = qk2[0:64, hp, 1, :]
            ckh = ck_f2[0:64, hp, :]
        else:
            qT = q_od[:, hp, :]
            kT = k_od[:, hp, :]
            nc.sync.dma_start(out=ck_lo[:, hp, :], in_=ck_f2[64:128, hp, :])
            ckh = ck_lo[:, hp, :]

        # cv to natural [block, dh] orientation (ones col already set)
        ps_cvt = psA.tile([DH, DH], BF16, name="ps_cvt", tag="psa")
        if par == 0:
            nc.tensor.transpose(ps_cvt[:], cv_f2[0:64, hp, :],
                                ident[0:64, 0:64])
        else:
            nc.tensor.transpose(ps_cvt[:], cv_f2[64:128, hp, :],
                                ident[64:128, 64:128])
        nc.scalar.copy(cv_aug[:, hb, 0:DH], ps_cvt[:])

        # -- compressed branch ------------------------------------------
        ps_oc = psO.tile([DH + 1, N], F32, name="ps_oc")
        for ch in range(2):
            sl = slice(512 * ch, 512 * (ch + 1))
            ps_sc = psA.tile([NB, 512], F32, name="ps_sc", tag="psa")
            mm(ps_sc[:], ckh, qT[:, sl], start=True, stop=True)
            nc.vector.tensor_tensor(ps_sc[:], ps_sc[:], cmask[:, sl], op=AL.add)
            nc.scalar.activation(pc[:, hb, sl], ps_sc[:], AF.Exp, scale=SCALE)
            mm(ps_oc[:, sl], cv_aug[:, hb, :], pc[:, hb, sl],
               start=True, stop=True)

        # -- sliding window branch ----------------------------------------
        for g in range(8):
            nq = 192 if g < 7 else 128
            ps_sw = psA.tile([128, 192], F32, name="ps_sw", tag="psa")
            mm(ps_sw[:, :nq], kT[:, 128 * g:128 * (g + 1)],
               qT[:, 128 * g:128 * g + nq], start=True, stop=True)
            nc.vector.tensor_tensor(ps_sw[:, :nq], ps_sw[:, :nq],
                                    wmask[:, :nq], op=AL.add)
            nc.scalar.activation(pw[:, hb, g, 0:nq], ps_sw[:, :nq], AF.Exp,
                                 scale=SCALE)

        # window P@V: even key tiles cover query spans [256g0, 256g0+256),
        # odd tiles cover [128+256g, ...) in a second, 128-shifted psum.
        wsel = (lambda g: vnat2[:, hp, g, 0:65]) if par == 0 else \
               (lambda g: vnat2[:, hp, g, 65:130])
        ps_owA = psO.tile([DH + 1, N], F32, name="ps_owA")
        ps_owB = psO.tile([DH + 1, N], F32, name="ps_owB")
        for g in (0, 2, 4, 6):
            mm(ps_owA[:, 128 * g:128 * g + 256], wsel(g), pw[:, hb, g, 0:256],
               start=True, stop=True)
        for g in (1, 3, 5):
            mm(ps_owB[:, 128 * (g - 1):128 * (g - 1) + 256], wsel(g),
               pw[:, hb, g, 0:256], start=True, stop=True)
        mm(ps_owB[:, 768:896], wsel(7), pw[:, hb, 7, 0:128],
           start=True, stop=True)

        # -- denominators, gates, mix -------------------------------------
        nc.scalar.copy(dw64[64:65, 0:128], ps_owA[64:65, 0:128])
        nc.scalar.copy(dwbs[64:65, :], ps_owB[64:65, 0:896])
        nc.vector.tensor_tensor(dw64[64:65, 128:N], ps_owA[64:65, 128:N],
                                dwbs[64:65, :], op=AL.add)
        # reciprocal_approx_fast misbehaves at partition offset 64: shift the
        # two denominator rows down to partitions 0/1 and batch one recip.
        nc.scalar.copy(dc64[64:65, :], ps_oc[64:65, :])
        nc.sync.dma_start(out=d2[0:1, :], in_=dc64[64:65, :])
        nc.sync.dma_start(out=d2[1:2, :], in_=dw64[64:65, :])
        nc.vector.reciprocal_approx_fast(out=r2[:], in_=d2[:])
        nc.vector.tensor_tensor(g2[:], r2[:], w3_sb[0:2, :], op=AL.mult)
        # tokens 0..14 see no compressed block: den==0 -> force gate to 0
        nc.vector.memset(g2[0:1, 0:15], 0.0)
        nc.sync.dma_start(out=gw_row[:], in_=g2[1:2, :])
        nc.gpsimd.partition_broadcast(gc_b[:], g2[0:1, :])
        nc.gpsimd.partition_broadcast(gw_b[:], gw_row[:])
        if DEBUG and h == 0:
            nc.sync.dma_start(out=dbg["den"].ap(), in_=r2[:])
            nc.sync.dma_start(out=dbg["denr"].ap(), in_=d2[:])

        nc.vector.tensor_tensor(t1[:], ps_oc[0:DH, :], gc_b[:], op=AL.mult)
        nc.vector.tensor_tensor(t2[:], ps_owA[0:DH, :], gw_b[:], op=AL.mult)
        nc.vector.tensor_tensor(comb[:, h, :], t1[:], t2[:], op=AL.add)
        nc.vector.tensor_tensor(t3[:], ps_owB[0:DH, 0:896],
                                gw_b[:, 128:N], op=AL.mult)
        nc.vector.tensor_tensor(comb[:, h, 128:N], comb[:, h, 128:N],
                                t3[:], op=AL.add)

        if DEBUG and h == 0:
            nc.sync.dma_start(out=dbg["ck"].ap(), in_=ck_f2[:, hp, :])
            nc.sync.dma_start(out=dbg["cva"].ap(), in_=cv_aug[:, hb, :])
        if DEBUG and h == 1:
            nc.sync.dma_start(out=dbg["pc"].ap(), in_=pc[:])
            nc.sync.dma_start(out=dbg["pw"].ap(), in_=pw[:])
            nc.sync.dma_start(out=dbg["vnat"].ap(), in_=vnat2[:])

        # -- per-head AllGather chunk ------------------------------------
        nc.sync.dma_start(out=cc_in[64 * h:64 * (h + 1), :], in_=comb[:, h, :])
        nc.gpsimd.collective_compute(
            "AllGather", AL.bypass, replica_groups=GROUPS,
            ins=[cc_in[64 * h:64 * (h + 1), :].opt()],
            outs=[cc_out[256 * h:256 * (h + 1), :].opt()])

    if DEBUG:
        nc.sync.dma_start(out=dbg["comb"].ap(), in_=comb[:])

    psO_cm.__exit__(None, None, None)
    psA_cm.__exit__(None, None, None)

    # ----- stage 7: output projection over gathered head chunks ----------
    cmb, cmb_free = tc.tile([128, HPC, 2, N], BF16, name="cmb")
    outT_sb, outT_sb_free = tc.tile([128, 2, N], F32, name="outT_sb")
    for h in range(HPC):
        for s in range(2):
            nc.gpsimd.dma_start(
                out=cmb[:, h, s, :],
                in_=cc_out[256 * h + 128 * s:256 * h + 128 * (s + 1), :])
    if DEBUG:
        nc.sync.dma_start(out=dbg["cmb"].ap(), in_=cmb[:])

    psW_cm = tc.tile_pool(name="psW", bufs=4, space="PSUM")
    psW = psW_cm.__enter__()
    po = {}
    for m in range(2):
        for ch in range(2):
            po[(m, ch)] = psW.tile([128, 512], F32, name=f"po{m}{ch}", bufs=1)
    for h in range(HPC):
        for s in range(2):
            for m in range(2):
                for ch in range(2):
                    sl = slice(512 * ch, 512 * (ch + 1))
                    mm(po[(m, ch)][:], wout_sb[:, h, s, 128 * m:128 * (m + 1)],
                       cmb[:, h, s, sl], start=(h == 0 and s == 0),
                       stop=(h == HPC - 1 and s == 1))
    for m in range(2):
        for ch in range(2):
            sl = slice(512 * ch, 512 * (ch + 1))
            nc.scalar.copy(outT_sb[:, m, sl], po[(m, ch)][:])
    nc.sync.dma_start(out=outT_d.ap().rearrange("(m p) n -> p m n", p=128),
                      in_=outT_sb[:])

    psW_cm.__exit__(None, None, None)
    outT_sb_free()
    cmb_free()
    dram_cm.__exit__(None, None, None)
    att_cm.__exit__(None, None, None)
    k_od_free()
    q_od_free()
    v2_free()
    qk2_free()
    const_cm.__exit__(None, None, None)


# --------------------------------------------------------------------------
_CACHE: dict = {}


def _get_nc() -> bass.Bass:
    if "nc" not in _CACHE:
        _CACHE["nc"] = build_program()
    return _CACHE["nc"]


def _prep_core(c: int, inputs: dict) -> dict:
    b, r = c // 4, c % 4
    hs = HPC * r
    f32, f64 = np.float32, np.float64
    inp = np.asarray(inputs["inp"], f32)
    rms_w = np.asarray(inputs["rms_w"], f32)
    Wqkv = np.asarray(inputs["Wqkv"], f32)
    k_pos = np.asarray(inputs["k_pos"], f32)
    v_pos = np.asarray(inputs["v_pos"], f32)
    k_cw = np.asarray(inputs["k_cw"], f32)
    k_cb = np.asarray(inputs["k_cb"], f32)
    v_cw = np.asarray(inputs["v_cw"], f32)
    v_cb = np.asarray(inputs["v_cb"], f32)
    Ws = np.asarray(inputs["Ws"], f32)
    bs = np.asarray(inputs["bs"], f32)
    Wout = np.asarray(inputs["Wout"], f32)

    # rms_w folded into the projection weights
    cols = [Wqkv[:, p * H * DH + hs * DH: p * H * DH + (hs + HPC) * DH]
            for p in range(3)]
    w_all = np.concatenate(cols + [Ws], axis=1) * rms_w[:, None].astype(f64)
    w_all = np.ascontiguousarray(w_all).astype(NPB)

    # conv weights: per head pair, block-diagonal [64*ha + i, pair, t, 64*hb + o]
    def conv_pack(cw):
        t = np.zeros((128, 2, CB, 128), f32)
        for pair in range(2):
            for half in range(2):
                blk = cw[hs + 2 * pair + half].transpose(1, 2, 0)  # [i, t, o]
                t[64 * half:64 * half + 64, pair, :,
                  64 * half:64 * half + 64] = blk
        return np.ascontiguousarray(t).astype(NPB)

    # conv(pos) folded into the conv bias: bias_eff packed [64*hb + o, pair]
    def bias_fold(cb, cw, pos):
        cp = np.einsum("hoit,hti->ho", cw[hs:hs + HPC].astype(f64),
                       pos[hs:hs + HPC].astype(f64))
        be = cb[hs:hs + HPC] + cp                        # [4, o]
        out = np.zeros((128, 2), f32)
        for pair in range(2):
            for half in range(2):
                out[64 * half:64 * (half + 1), pair] = be[2 * pair + half]
        return np.ascontiguousarray(out)

    # output projection slabs: chunk h delivers global heads (8s+h, 8s+4+h)
    woutS = np.zeros((128, HPC, 2, 256), f32)
    for h in range(HPC):
        for s in range(2):
            ga, gb = 8 * s + h, 8 * s + 4 + h
            woutS[0:64, h, s, :] = Wout[64 * ga:64 * ga + 64,
                                        256 * r:256 * (r + 1)]
            woutS[64:128, h, s, :] = Wout[64 * gb:64 * gb + 64,
                                          256 * r:256 * (r + 1)]

    return {
        "inpT": np.ascontiguousarray(inp[b].T).astype(NPB),
        "w_all": w_all,
        "cw_k": conv_pack(k_cw),
        "cw_v": conv_pack(v_cw),
        "kcb": bias_fold(k_cb, k_cw, k_pos),
        "vcb": bias_fold(v_cb, v_cw, v_pos),
        "bs_t": np.ascontiguousarray(bs[:, None]),
        "woutS": woutS.astype(NPB),
        "ones_c": np.ones((128, 8), NPB),
        "ident_c": np.eye(128, dtype=NPB),
    }


def kernel(**inputs) -> np.ndarray:
    nc = _get_nc()
    in_maps = [_prep_core(c, inputs) for c in range(NCORES)]
    res = run_bass_kernel_spmd(nc, in_maps, list(range(NCORES)))
    out = np.zeros((B, N, DIM), np.float32)
    for c in range(NCORES):
        b, r = c // 4, c % 4
        out[b, :, 256 * r:256 * (r + 1)] = res.results[c]["outT"].T
    return out
